# revision 16
# baseline (speedup 1.0000x reference)
"""Trainium2 Bass kernel: multi-head self-attention (B=2, T=2048, D=1024, H=16),
8-core SPMD. Accepts FULL inputs, returns the FULL output.

Sharding: data-parallel over batch (2) x tensor-parallel over heads (4 groups
of 4). Each core computes attention for its 4 heads of one batch plus its
partial output projection; the host sums the 4 partials per batch (plus the
bias terms, folded exactly).

vC: vB + per-query-block normalization scheduled right behind each block
(descending hp1 so the projection drains during attention), all norm-path
intermediates uniquely tagged.
"""
import sys
if '/opt/trn_rl_repo' not in sys.path:
    sys.path.insert(0, '/opt/trn_rl_repo')
import numpy as np
import ml_dtypes
import concourse.bass as bass
import concourse.mybir as mybir
from concourse import bacc
from concourse.tile import TileContext

F32 = mybir.dt.float32
BF16 = mybir.dt.bfloat16
AL = mybir.AluOpType
EXP = mybir.ActivationFunctionType.Exp
BF = ml_dtypes.bfloat16

T = 2048
DM = 1024
HPC = 4
D = 64
NQB = 4           # query blocks of 512
NKC = 16          # key chunks of 128
NDC = 8           # contraction chunks of 128 for projections
LAG = 3           # PV lags S/exp by this many key chunks


def build_nc():
    nc = bacc.Bacc("TRN2", target_bir_lowering=False, debug=True)

    xp = nc.dram_tensor("xp", [128, NDC, T], BF16, kind="ExternalInput")
    wqk = nc.dram_tensor("wqk", [128, NDC, 512], BF16, kind="ExternalInput")
    wv = nc.dram_tensor("wv", [128, NDC, 260], BF16, kind="ExternalInput")
    wp = nc.dram_tensor("wp", [128, 2, DM], BF16, kind="ExternalInput")
    msk = nc.dram_tensor("msk", [128, 1024], BF16, kind="ExternalInput")
    y = nc.dram_tensor("y", [T, DM], F32, kind="ExternalOutput")

    with nc.allow_low_precision("bf16 matmul pipeline"), TileContext(nc) as tc:
        from contextlib import ExitStack
        ctx = ExitStack()
        cp = ctx.enter_context(tc.tile_pool(name="const", bufs=1))
        wtp = ctx.enter_context(tc.tile_pool(name="wts", bufs=1))
        qkvp = ctx.enter_context(tc.tile_pool(name="qkv", bufs=1))
        psS = ctx.enter_context(tc.tile_pool(name="psS", bufs=2, space="PSUM"))
        psO = ctx.enter_context(tc.tile_pool(name="psO", bufs=1, space="PSUM"))
        psX = ctx.enter_context(tc.tile_pool(name="psX", bufs=2, space="PSUM"))

        wqk_t = wtp.tile([128, NDC * 512], BF16, tag="wqk", name="wqk")
        wv_t = wtp.tile([128, NDC * 260], BF16, tag="wv", name="wv")
        wp_t = wtp.tile([128, 2 * DM], BF16, tag="wp", name="wp")
        mask_t = cp.tile([128, 1024], BF16, tag="m0", name="m0")
        ones_t = cp.tile([128, 64], BF16, tag="ones", name="ones")
        nc.vector.memset(ones_t[:], 1.0)
        onesv = cp.tile([128, 4], BF16, tag="onesv", name="onesv")
        nc.vector.memset(onesv[:], 1.0)

        # persistent activations
        QT = [qkvp.tile([128, T], BF16, tag=f"qt{i}", name=f"qt{i}") for i in range(2)]
        KT = [qkvp.tile([128, T], BF16, tag=f"kt{i}", name=f"kt{i}") for i in range(2)]
        V = [qkvp.tile([128, 260], BF16, tag=f"v{t}", name=f"v{t}") for t in range(NKC)]
        OTS = [qkvp.tile([128, T], BF16, tag=f"ots{j}", name=f"ots{j}")
               for j in range(2)]

        def wq_sl(k, fc):
            return wqk_t[:, k * 512 + fc * 128:k * 512 + fc * 128 + 128]

        def wk_sl(k, fc):
            return wqk_t[:, k * 512 + 256 + fc * 128:k * 512 + 256 + fc * 128 + 128]

        def wv_sl(k):
            return wv_t[:, k * 260:(k + 1) * 260]

        def wp_sl(jc, mb):
            return wp_t[:, jc * DM + mb * 512:jc * DM + mb * 512 + 512]

        # ---------------- phase B: fc0 QKV projections ----------------
        xtp = ctx.enter_context(tc.tile_pool(name="xt", bufs=1))
        xt2 = [xtp.tile([128, 2 * T], BF16, tag=f"xa{i}", name=f"xa{i}")
               for i in range(4)]

        def xt_sl(k, c0, c1):
            return xt2[k // 2][:, (k % 2) * T + c0:(k % 2) * T + c1]

        nc.sync.dma_start(wqk_t[:], wqk[:, :, :])
        for k2 in range(4):
            nc.sync.dma_start(xt2[k2][:], xp[:, 2 * k2:2 * k2 + 2, :])
        nc.sync.dma_start(wv_t[:], wv[:, :, :])
        nc.sync.dma_start(wp_t[:], wp[:, :, :])
        nc.sync.dma_start(mask_t[:], msk[:, :])

        def qk_group(is_k, fc, tb, eng):
            OUT = KT if is_k else QT
            ps = psX.tile([128, 512], F32, tag="b", name="qkps")
            for k in range(NDC):
                w = wk_sl(k, fc) if is_k else wq_sl(k, fc)
                nc.tensor.matmul(
                    ps[:], w, xt_sl(k, tb * 512, (tb + 1) * 512),
                    start=(k == 0), stop=(k == NDC - 1))
            if eng == "act":
                nc.scalar.copy(OUT[fc][:, tb * 512:(tb + 1) * 512], ps[:])
            else:
                nc.vector.tensor_copy(OUT[fc][:, tb * 512:(tb + 1) * 512], ps[:])

        def v_tile(tt, eng="dve"):
            ps = psX.tile([128, 260], F32, tag="b", name="vps")
            for k in range(NDC):
                nc.tensor.matmul(
                    ps[:], xt_sl(k, tt * 128, (tt + 1) * 128), wv_sl(k),
                    start=(k == 0), stop=(k == NDC - 1))
            if eng == "act":
                nc.scalar.copy(V[tt][:], ps[:])
            else:
                nc.vector.tensor_copy(V[tt][:], ps[:])
            nc.vector.tensor_copy(
                V[tt].rearrange("p (h c) -> p h c", c=65)[:, :, 64:65],
                onesv[:].rearrange("p (h c) -> p h c", c=1))

        for is_k in (False, True):
            for tb in range(4):
                qk_group(is_k, 0, tb, "act" if tb % 2 == 0 else "dve")
        for tt in range(4):
            v_tile(tt, "act")

        # ---------------- phase C: attention ----------------
        ptp = ctx.enter_context(tc.tile_pool(name="pt", bufs=5))
        rcp = ctx.enter_context(tc.tile_pool(name="rcp", bufs=2))
        ybp = ctx.enter_context(tc.tile_pool(name="yb", bufs=3))

        dd_all = {}   # (hp, hh, qb) -> [1, 512] bf16 reciprocal denominators
        ou_all = {}   # (hp, hh, qb) -> [65, 512] f32 unnormalized O (+denom)

        def recip_unit(hp, hh, qb):
            def emit():
                dh = dd_all[("dh", hp, hh, qb)]
                dhr = rcp.tile([1, 512], F32, tag="dhr",
                               name="dhr", bufs=2)
                nc.vector.reciprocal_approx_fast(dhr[:], dh[:])
                dd = rcp.tile([1, 512], BF16, tag=f"dd{hh}",
                              name=f"dd{hh}", bufs=4)
                nc.vector.tensor_copy(dd[:], dhr[:])
                dd_all[(hp, hh, qb)] = dd
            return emit

        def norm_unit(hp, hh, qb):
            def emit():
                psb = psX.tile([64, 512], F32, tag="b", name="nps")
                nc.tensor.matmul(
                    psb[:], ones_t[0:1, 0:64], dd_all[(hp, hh, qb)][:],
                    start=True, stop=True)
                ou = ou_all[(hp, hh, qb)]
                if hh == 0:
                    nc.vector.tensor_tensor(
                        OTS[hp][0:64, qb * 512:(qb + 1) * 512],
                        ou[0:64, :], psb[:], AL.mult)
                else:
                    ob = rcp.tile([64, 512], BF16, tag="ob",
                                  name="ob", bufs=3)
                    nc.vector.tensor_tensor(
                        ob[:], ou[0:64, :], psb[:], AL.mult)
                    nc.sync.dma_start(
                        OTS[hp][64:128, qb * 512:(qb + 1) * 512], ob[:])
            return emit

        ybt = {}

        def proj_unit(tt, mb):
            def emit():
                psy = psX.tile([128, 512], F32, tag="b", name="yps")
                for jc in range(2):
                    nc.tensor.matmul(
                        psy[:], OTS[jc][:, tt * 128:(tt + 1) * 128],
                        wp_sl(jc, mb), start=(jc == 0), stop=(jc == 1))
                yt = ybp.tile([128, 512], F32, tag="yt", name="yt")
                if (tt + mb) % 2 == 0:
                    nc.scalar.copy(yt[:], psy[:])
                else:
                    nc.vector.tensor_copy(yt[:], psy[:])
                nc.sync.dma_start(
                    y[tt * 128:(tt + 1) * 128, mb * 512:(mb + 1) * 512],
                    yt[:])
            return emit

        # fc1 Q/K groups and V tiles 4..15, injected into hp0's stream
        inj0 = [lambda tt=tt: v_tile(tt) for tt in range(4, 8)]
        for g in range(4):
            inj0 += [lambda g=g: qk_group(False, 1, g, "dve")]
            inj0 += [lambda tt=tt: v_tile(tt) for tt in (8 + g,)]
        inj0 += [lambda tt=tt: v_tile(tt) for tt in (12,)]
        for g in range(4):
            inj0 += [lambda g=g: qk_group(True, 1, g, "dve")]
            if 13 + g < 16:
                inj0 += [lambda tt=tt: v_tile(tt) for tt in (13 + g,)]
        inject = list(inj0)

        for hp in range(HPC // 2):
            fc = hp
            heads = (2 * hp, 2 * hp + 1)
            qb_order = list(range(NQB)) if hp == 0 else [3, 2, 1, 0]
            for qb in qb_order:
                nkc = 4 * (qb + 1)
                pso = {h: psO.tile([65, 512], F32, tag=f"o{h % 2}",
                                   name=f"o{h % 2}") for h in heads}
                ptq = {}
                offs = {}
                for kc in range(nkc + LAG):
                    if kc < nkc:
                        t = kc - 4 * qb
                        off = 128 * t if t > 0 else 0
                        w = 512 - off
                        pss = psS.tile([128, 1024], F32, tag="s", name="s")
                        for h in heads:
                            po = 64 * (h % 2)
                            nc.tensor.matmul(
                                pss[:, po * 8 + off:po * 8 + 512],
                                KT[fc][po:po + 64, kc * 128:(kc + 1) * 128],
                                QT[fc][po:po + 64,
                                       qb * 512 + off:(qb + 1) * 512],
                                start=True, stop=True)
                        pt = ptp.tile([128, 1024], BF16, tag="pt", name="pt")
                        if off == 0:
                            nc.scalar.activation(pt[:], pss[:], EXP)
                        else:
                            nc.scalar.activation(
                                pt[:].rearrange("p (h q) -> p h q", h=2)[:, :, off:],
                                pss[:].rearrange("p (h q) -> p h q", h=2)[:, :, off:],
                                EXP)
                        if t >= 0:  # diagonal chunk -> causal mask
                            nc.vector.tensor_tensor(
                                pt[:].rearrange("p (h q) -> p h q", h=2)[:, :, off:],
                                pt[:].rearrange("p (h q) -> p h q", h=2)[:, :, off:],
                                mask_t[:].rearrange("p (h q) -> p h q", h=2)[:, :, 0:w],
                                AL.mult)
                        ptq[kc] = pt
                        offs[kc] = off
                    kcp = kc - LAG
                    if kcp >= 0 and kcp in ptq:
                        ptv = ptq.pop(kcp)
                        off2 = offs.pop(kcp)
                        for h in heads:
                            po = 64 * (h % 2)
                            nc.tensor.matmul(
                                pso[h][:, off2:],
                                V[kcp][:, 65 * h:65 * h + 65],
                                ptv[:, po * 8 + off2:po * 8 + 512],
                                start=(kcp == 0),
                                stop=(kcp == nkc - 1),
                                skip_group_check=True)
                    if kcp >= 0:
                        npop = 1 if hp == 0 else (3 if len(inject) >= 12 else 2)
                        for _ in range(npop):
                            if inject:
                                inject.pop(0)()
                # query-block epilogue: copy out unnormalized O; gather the
                # denominator row into partition 0 for the per-qb reciprocal
                for h in heads:
                    hh = h % 2
                    ou = rcp.tile([65, 512], F32, tag=f"ou{hh}",
                                  name=f"ou{hh}", bufs=4)
                    nc.vector.tensor_copy(ou[:], pso[h][:])
                    ou_all[(hp, hh, qb)] = ou
                    dh = rcp.tile([1, 512], F32, tag=f"dh{hh}",
                                  name=f"dh{hh}", bufs=4)
                    nc.sync.dma_start(dh[:], ou[64:65, :])
                    dd_all[("dh", hp, hh, qb)] = dh
                inject += [recip_unit(hp, hh, qb) for hh in (0, 1)]
                inject += [norm_unit(hp, hh, qb) for hh in (0, 1)]
                if hp == 1:
                    inject += [proj_unit(tt, mb)
                               for tt in range(4 * qb, 4 * qb + 4)
                               for mb in (0, 1)]
        # drain
        while inject:
            inject.pop(0)()
        ctx.close()

    nc.finalize()
    return nc


def make_mask():
    m = np.zeros((128, 1024), dtype=np.float32)
    f = np.arange(512)
    p = np.arange(128)
    pat = (f[None, :] >= p[:, None]).astype(np.float32)
    m[:, 0:512] = pat
    m[:, 512:1024] = pat
    return m.astype(BF)


def _pack(a, inner):
    """[1024, inner] -> [128, 8, inner] with [p, k, :] = a[k*128+p, :]."""
    return np.ascontiguousarray(
        a.reshape(NDC, 128, inner).transpose(1, 0, 2))


def shard_inputs(x, Wqkv, bqkv, Wproj):
    x = np.asarray(x, dtype=np.float32)
    Wqkv = np.asarray(Wqkv, dtype=np.float32)
    bqkv = np.asarray(bqkv, dtype=np.float32)
    Wproj = np.asarray(Wproj, dtype=np.float32)
    assert not np.any(bqkv[0:2048]), \
        "nonzero q/k bias not supported by the fast kernel"
    mask = make_mask()
    in_maps = []
    for c in range(8):
        b, g = c // 4, c % 4
        cs = slice(256 * g, 256 * g + 256)
        wq_ = Wqkv[:, 0:1024][:, cs] / 8.0
        wk_ = Wqkv[:, 1024:2048][:, cs]
        wqk_ = np.concatenate([wq_, wk_], axis=1)  # [1024, 512]
        wv_src = Wqkv[:, 2048:3072][:, cs]
        wv_ = np.zeros((DM, 260), dtype=np.float32)
        for h in range(4):
            wv_[:, 65 * h:65 * h + 64] = wv_src[:, 64 * h:64 * h + 64]
        wp_ = np.ascontiguousarray(
            Wproj[256 * g:256 * g + 256, :].reshape(2, 128, DM)
            .transpose(1, 0, 2))
        in_maps.append({
            "xp": _pack(x[b].T, T).astype(BF),
            "wqk": _pack(wqk_, 512).astype(BF),
            "wv": _pack(wv_, 260).astype(BF),
            "wp": wp_.astype(BF),
            "msk": mask,
        })
    return in_maps


def combine_outputs(results, Wqkv, bqkv, Wproj, bproj):
    bqkv = np.asarray(bqkv, dtype=np.float32)
    Wproj = np.asarray(Wproj, dtype=np.float32)
    bproj = np.asarray(bproj, dtype=np.float32)
    bv_term = bqkv[2048:3072] @ Wproj
    out = np.zeros((2, T, DM), dtype=np.float32)
    for c in range(8):
        out[c // 4] += results[c]["y"].reshape(T, DM)
    out += (bv_term + bproj)[None, None, :]
    return out


_NC_CACHE = []


def _numpy_fallback(x, Wqkv, bqkv, Wproj, bproj):
    b, t, dm = x.shape
    h, d = 16, 64
    qkv = x @ Wqkv + bqkv
    q, k, v = np.split(qkv, 3, axis=-1)
    q = q.reshape(b, t, h, d).transpose(0, 2, 1, 3)
    k = k.reshape(b, t, h, d).transpose(0, 2, 1, 3)
    v = v.reshape(b, t, h, d).transpose(0, 2, 1, 3)
    att = np.einsum('bhqd,bhkd->bhqk', q, k) / np.sqrt(np.float32(d))
    causal = np.tril(np.ones((t, t), dtype=bool))
    att = np.where(causal[None, None], att, -np.inf)
    att = att - att.max(axis=-1, keepdims=True)
    e = np.exp(att)
    p = e / e.sum(axis=-1, keepdims=True)
    out = np.einsum('bhqk,bhkd->bhqd', p, v)
    out = out.transpose(0, 2, 1, 3).reshape(b, t, dm)
    return (out @ Wproj + bproj).astype(np.float32)


def kernel(x, Wqkv, bqkv, Wproj, bproj):
    x = np.asarray(x, dtype=np.float32)
    Wqkv = np.asarray(Wqkv, dtype=np.float32)
    bqkv = np.asarray(bqkv, dtype=np.float32)
    Wproj = np.asarray(Wproj, dtype=np.float32)
    bproj = np.asarray(bproj, dtype=np.float32)
    if np.any(bqkv[0:2048]):
        return _numpy_fallback(x, Wqkv, bqkv, Wproj, bproj)
    from concourse.bass_utils import run_bass_kernel_spmd
    if not _NC_CACHE:
        _NC_CACHE.append(build_nc())
    nc = _NC_CACHE[0]
    in_maps = shard_inputs(x, Wqkv, bqkv, Wproj)
    res = run_bass_kernel_spmd(nc, in_maps, core_ids=list(range(8)))
    return combine_outputs(res.results, Wqkv, bqkv, Wproj, bproj)
